# revision 6
# baseline (speedup 1.0000x reference)
"""Trainium2 Bass kernel for BlockAutoregressiveLinear (n_blocks=128, B=32,
in=out=4096, batch=2048), 8 NeuronCores.

Math (see reference):
    Wm = exp(W)*diag_mask + W*tril_mask          (block-diag exp, strict block-lower copy)
    s[o] = sum_i Wm[o,i]^2
    g[o] = exp(W_log_scale[o]) / sqrt(s[o]) = exp(W_log_scale[o] - 0.5*ln s[o])
    y = x @ (g[:,None]*Wm).T + bias
    log_jac[k,a,b] = log(g * exp(W))[diag block k] = W[k*32+a, k*32+b] + W_log_scale[o] - 0.5*ln s[o]

Sharding: 4 column groups x 2 batch halves. Core c=(2g+h) owns 8 output
m-tiles (128 rows each) mg = 4s+g for slot s=0..7 and batch half h.
The weight matrix is block-lower-triangular, so m-tile mg only needs
k-tiles 0..mg.  For an SPMD-uniform instruction stream across all 8
cores, slot s is padded to K_s = 4(s+1) k-tile matmuls (the pad tiles
are host-packed zeros); every core then executes the identical program
and all per-core variation lives in the input data.

The diagonal-block exp contribution is handled as a separate per-slot
matmul (a_exp tile built on device with ACT Exp) against a host-packed
copy of the corresponding x rows (x_mixed), so the stream tiles are pure
copies of W.T (masked on host by zeroing, which is data selection only —
all arithmetic runs on device).
"""

import sys

sys.path.insert(0, "/opt/trn_rl_repo")

import numpy as np

NB = 128          # autoregressive blocks
BLK = 32          # block size
DIM = 4096
BATCH = 2048
NSLOT = 8         # m-tile slots per core
NGRP = 4          # column groups
NT = 4            # n-tiles per batch half
NTW = 256         # n-tile width
KT = 32           # k tiles

# k-major stream of (kt, slot) jobs; slot s is active for kt < 4(s+1)
STREAM = [(kt, s) for kt in range(KT) for s in range(NSLOT) if kt < 4 * (s + 1)]
T_IDX = {j: t for t, j in enumerate(STREAM)}
NSTREAM = len(STREAM)  # 144


def _kt_range(kt):
    """[t0, t1) tile-index range of stream tiles belonging to k-tile kt."""
    t0 = T_IDX[(kt, kt // 4)]
    nact = NSLOT - kt // 4
    return t0, t0 + nact


def _build_program():
    import concourse.tile as tile
    from concourse import bacc, mybir

    F32 = mybir.dt.float32
    F32R = mybir.dt.float32r
    AF = mybir.ActivationFunctionType

    nc = bacc.Bacc("TRN2", target_bir_lowering=False, debug=False, num_devices=8)

    ws = nc.dram_tensor("ws", [128, NSTREAM * 128], F32R, kind="ExternalInput")
    xt = nc.dram_tensor("xt", [DIM, 1024], F32R, kind="ExternalInput")
    xm = nc.dram_tensor("xm", [128, NSLOT, 1024], F32R, kind="ExternalInput")
    dgt = nc.dram_tensor("dgt", [128, NSLOT, BLK], F32, kind="ExternalInput")
    dgr = nc.dram_tensor("dgr", [128, NSLOT, BLK], F32, kind="ExternalInput")
    b8 = nc.dram_tensor("b8", [128, NSLOT], F32, kind="ExternalInput")
    ls8 = nc.dram_tensor("ls8", [128, NSLOT], F32, kind="ExternalInput")
    on1 = nc.dram_tensor("on1", [128, 1], F32R, kind="ExternalInput")
    zz = nc.dram_tensor("zz", [128, NSLOT * 128], F32R, kind="ExternalInput")
    yt = nc.dram_tensor("yt", [1024, 1024], F32, kind="ExternalOutput")
    lj = nc.dram_tensor("lj", [128, NSLOT, BLK], F32, kind="ExternalOutput")
    srt = nc.dram_tensor("srt", [1024], F32)  # s roundtrip scratch

    with tile.TileContext(nc) as tc:
        with (
            tc.tile_pool(name="const", bufs=1) as const,
            tc.tile_pool(name="sqp", bufs=3) as sqp,
            tc.tile_pool(name="xp", bufs=2) as xp,
            tc.tile_pool(name="xmp", bufs=2) as xmp,
            tc.tile_pool(name="yp", bufs=4) as yp,
            tc.tile_pool(name="smallp", bufs=1) as smallp,
        ):
            # ---- W stream + constants ----
            w_all = const.tile([128, NSTREAM * 128], F32R)
            for kt in range(KT):
                t0, t1 = _kt_range(kt)
                nc.sync.dma_start(
                    out=w_all[:, t0 * 128 : t1 * 128], in_=ws[:, t0 * 128 : t1 * 128]
                )
            ones = const.tile([128, 1], F32R)
            nc.sync.dma_start(out=ones, in_=on1[:, :])
            dgt_sb = smallp.tile([128, NSLOT, BLK], F32)
            nc.sync.dma_start(out=dgt_sb, in_=dgt[:, :, :])
            dgr_sb = smallp.tile([128, NSLOT, BLK], F32)
            nc.sync.dma_start(out=dgr_sb, in_=dgr[:, :, :])
            b8_sb = smallp.tile([128, NSLOT], F32)
            nc.sync.dma_start(out=b8_sb, in_=b8[:, :])
            ls8_sb = smallp.tile([128, NSLOT], F32)
            nc.sync.dma_start(out=ls8_sb, in_=ls8[:, :])

            # ---- diag-block exp tiles (lhsT orientation) ----
            a_exp = const.tile([128, NSLOT, 128], F32R)
            nc.sync.dma_start(out=a_exp.rearrange("p s f -> p (s f)"), in_=zz[:, :])
            for s in range(NSLOT):
                for j in range(4):
                    nc.scalar.activation(
                        a_exp[32 * j : 32 * j + 32, s, 32 * j : 32 * j + 32],
                        dgt_sb[32 * j : 32 * j + 32, s, :],
                        AF.Exp,
                    )

            # ---- row norms: s[o] = column sums of squares (ones-matmul) ----
            with tc.tile_pool(name="psn", bufs=1, space="PSUM") as pnp:
                psn = pnp.tile([1, 1024], F32)
                for kt in range(KT):
                    t0, t1 = _kt_range(kt)
                    nact = t1 - t0
                    w_slice = w_all[:, t0 * 128 : t1 * 128]
                    sq = sqp.tile([128, 1024], F32R, tag="sq")
                    nc.vector.tensor_mul(sq[:, : nact * 128], w_slice, w_slice)
                    smin = (kt // 4) * 128
                    off = 0
                    while off < nact * 128:
                        n = min(512, nact * 128 - off)
                        nc.tensor.matmul(
                            psn[0:1, smin + off : smin + off + n],
                            ones,
                            sq[:, off : off + n],
                            start=(kt == 0),
                            stop=False,
                        )
                        off += n
                a_sq = sqp.tile([128, 1024], F32R, tag="sq")
                ae_flat = a_exp.rearrange("p s f -> p (s f)")
                nc.vector.tensor_mul(a_sq, ae_flat, ae_flat)
                nc.tensor.matmul(psn[0:1, 0:512], ones, a_sq[:, 0:512], start=False, stop=False)
                nc.tensor.matmul(psn[0:1, 512:1024], ones, a_sq[:, 512:1024], start=False, stop=True)
                s_row = smallp.tile([1, 1024], F32)
                nc.scalar.copy(s_row, psn[0:1, :])

            # s roundtrip through DRAM to get per-partition layout [128, 8]
            nc.sync.dma_start(out=srt[:].rearrange("(a n) -> a n", a=1), in_=s_row[0:1, :])
            s_col = smallp.tile([128, NSLOT], F32)
            nc.sync.dma_start(out=s_col, in_=srt[:].rearrange("(s p) -> p s", p=128))

            # g = exp(ls - 0.5 ln s);  ljb = ls - 0.5 ln s
            t_ln = smallp.tile([128, NSLOT], F32)
            nc.scalar.activation(t_ln, s_col, AF.Ln)
            ljb = smallp.tile([128, NSLOT], F32)
            for s in range(NSLOT):
                nc.scalar.activation(
                    ljb[:, s : s + 1], t_ln[:, s : s + 1], AF.Identity,
                    bias=ls8_sb[:, s : s + 1], scale=-0.5,
                )
            g_sb = smallp.tile([128, NSLOT], F32)
            nc.scalar.activation(g_sb, ljb, AF.Exp)

            # log-jacobian: lj = W_diag + (ls - 0.5 ln s)
            lj_sb = smallp.tile([128, NSLOT, BLK], F32)
            for s in range(NSLOT):
                nc.scalar.activation(
                    lj_sb[:, s, :], dgr_sb[:, s, :], AF.Identity,
                    bias=ljb[:, s : s + 1], scale=1.0,
                )
            nc.sync.dma_start(out=lj[:, :, :], in_=lj_sb)

            # ---- main matmul: yT[slot, batch] ----
            with tc.tile_pool(name="psy", bufs=4, space="PSUM") as pyp:
                for nt in range(NT):
                    xt_sb = xp.tile([128, KT, NTW], F32R, tag="xt")
                    for kt in range(KT):
                        nc.sync.dma_start(
                            out=xt_sb[:, kt, :],
                            in_=xt[kt * 128 : (kt + 1) * 128, nt * NTW : (nt + 1) * NTW],
                        )
                    xm_sb = xmp.tile([128, NSLOT, NTW], F32R, tag="xm")
                    nc.sync.dma_start(out=xm_sb, in_=xm[:, :, nt * NTW : (nt + 1) * NTW])
                    for s in range(NSLOT):
                        ps_y = pyp.tile([128, NTW], F32, tag="psy")
                        nc.tensor.matmul(
                            ps_y, a_exp[:, s, :], xm_sb[:, s, :], start=True, stop=False
                        )
                        for kt in range(4 * (s + 1)):
                            t = T_IDX[(kt, s)]
                            nc.tensor.matmul(
                                ps_y,
                                w_all[:, t * 128 : (t + 1) * 128],
                                xt_sb[:, kt, :],
                                start=False,
                                stop=(kt == 4 * s + 3),
                            )
                        y_sb = yp.tile([128, NTW], F32, tag="y")
                        nc.scalar.activation(
                            y_sb, ps_y, AF.Identity,
                            bias=b8_sb[:, s : s + 1], scale=g_sb[:, s : s + 1],
                        )
                        nc.sync.dma_start(
                            out=yt[s * 128 : (s + 1) * 128, nt * NTW : (nt + 1) * NTW],
                            in_=y_sb,
                        )
    nc.finalize()
    return nc


def _pack_inputs(x, W, bias, W_log_scale):
    """Host-side data layout: slice/transpose/zero-select only (no math)."""
    f32 = np.float32
    WT = np.ascontiguousarray(W.T.astype(f32, copy=False))
    xT = np.ascontiguousarray(x.T.astype(f32, copy=False))
    bias = bias.astype(f32, copy=False)
    ls = W_log_scale.astype(f32, copy=False)

    grp = {}
    for g in range(NGRP):
        ws_g = np.zeros((128, NSTREAM, 128), f32)
        for t, (kt, s) in enumerate(STREAM):
            mg = 4 * s + g
            if kt < mg:
                ws_g[:, t, :] = WT[kt * 128 : (kt + 1) * 128, mg * 128 : (mg + 1) * 128]
            elif kt == mg:
                blk = WT[kt * 128 : (kt + 1) * 128, mg * 128 : (mg + 1) * 128].copy()
                for jp in range(4):  # keep only sub-blocks strictly above the diagonal
                    blk[32 * jp : 32 * jp + 32, : 32 * (jp + 1)] = 0.0
                ws_g[:, t, :] = blk
            # kt > mg: zero pad
        dgt = np.empty((128, NSLOT, BLK), f32)
        dgr = np.empty((128, NSLOT, BLK), f32)
        for s in range(NSLOT):
            mg = 4 * s + g
            blk = W[mg * 128 : (mg + 1) * 128, mg * 128 : (mg + 1) * 128]
            for j in range(4):
                sub = blk[32 * j : 32 * j + 32, 32 * j : 32 * j + 32]  # [a, b]
                dgt[32 * j : 32 * j + 32, s, :] = sub.T
                dgr[32 * j : 32 * j + 32, s, :] = sub
        b8 = np.stack(
            [bias[(4 * s + g) * 128 : (4 * s + g + 1) * 128] for s in range(NSLOT)], 1
        )
        ls8 = np.stack(
            [ls[(4 * s + g) * 128 : (4 * s + g + 1) * 128, 0] for s in range(NSLOT)], 1
        )
        grp[g] = dict(
            ws=np.ascontiguousarray(ws_g.reshape(128, NSTREAM * 128)),
            dgt=dgt, dgr=dgr, b8=np.ascontiguousarray(b8), ls8=np.ascontiguousarray(ls8),
        )

    in_maps = []
    for c in range(8):
        g, h = c // 2, c % 2
        xt_h = np.ascontiguousarray(xT[:, h * 1024 : (h + 1) * 1024])
        xm_c = np.empty((128, NSLOT, 1024), f32)
        for s in range(NSLOT):
            mg = 4 * s + g
            xm_c[:, s, :] = xT[mg * 128 : (mg + 1) * 128, h * 1024 : (h + 1) * 1024]
        in_maps.append(
            {
                **grp[g],
                "xt": xt_h,
                "xm": xm_c,
                "on1": np.ones((128, 1), f32),
                "zz": np.zeros((128, NSLOT * 128), f32),
            }
        )
    return in_maps


_PROGRAM = None


def kernel(x, W, bias, W_log_scale, b_diag_mask=None, b_tril_mask=None, **_ignored):
    """Full inputs in, full outputs out. Returns (y, log_jac) like the reference.

    The diag/tril masks are the fixed block-kronecker patterns of the module
    (block-diagonal + strict block-lower-triangular); the kernel exploits that
    structure directly, so the mask tensors themselves are not transferred.
    """
    from concourse.bass_utils import run_bass_kernel_spmd

    global _PROGRAM
    if _PROGRAM is None:
        _PROGRAM = _build_program()
    nc = _PROGRAM

    in_maps = _pack_inputs(
        np.asarray(x), np.asarray(W), np.asarray(bias), np.asarray(W_log_scale)
    )
    res = run_bass_kernel_spmd(nc, in_maps, list(range(8)))

    y = np.empty((BATCH, DIM), np.float32)
    lj_full = np.empty((NB, BLK, BLK), np.float32)
    for c, r in enumerate(res.results):
        g, h = c // 2, c % 2
        ytc = r["yt"]
        for s in range(NSLOT):
            mg = 4 * s + g
            y[h * 1024 : (h + 1) * 1024, mg * 128 : (mg + 1) * 128] = (
                ytc[s * 128 : (s + 1) * 128, :].T
            )
        if h == 0:
            ljc = r["lj"]  # [128, 8, 32]
            for s in range(NSLOT):
                mg = 4 * s + g
                for j in range(4):
                    lj_full[4 * mg + j] = ljc[32 * j : 32 * j + 32, s, :]
    return (y, lj_full)


# revision 21
# speedup vs baseline: 1.3094x; 1.3094x over previous
"""Trainium2 Bass kernel for BlockAutoregressiveLinear (n_blocks=128, B=32,
in=out=4096, batch=2048), 8 NeuronCores.

Math (see reference):
    Wm = exp(W)*diag_mask + W*tril_mask          (block-diag exp, strict block-lower copy)
    s[o] = sum_i Wm[o,i]^2
    g[o] = exp(W_log_scale[o] - 0.5*ln s[o])
    y = x @ (g[:,None]*Wm).T + bias
    log_jac[k,a,b] = W[k*32+a, k*32+b] + W_log_scale[o] - 0.5*ln s[o]

Sharding: 4 column groups x 2 batch halves. Core c=(2g+h) owns 8 output
m-tiles (128 rows each) mg = 4s+g for slot s=0..7 and batch half h.
W is block-lower-triangular, so m-tile mg only needs k-tiles 0..mg. For an
SPMD-uniform instruction stream, slot s is padded to K_s = 4(s+1) k-tile
matmuls (pad tiles are host-packed zeros); every core executes the identical
program and all per-core variation lives in the input data.

The diagonal-block exp contribution is a separate per-slot matmul (a_exp
tile built on device with ACT Exp) against a host-packed copy of the
corresponding x rows (xm), so the stream tiles are pure copies of W.T
(masked on host by zeroing = data selection; all arithmetic is on device).

Matmuls run in float32r (fp32 truncated to FP22 in the PE) at full rate
with N=512 moving tiles. The weight scaling by g and the bias add are fused
into the PSUM->SBUF copyback on the ACT engine (per-partition scale/bias).
"""

import sys

sys.path.insert(0, "/opt/trn_rl_repo")

import numpy as np

NB = 128          # autoregressive blocks
BLK = 32          # block size
DIM = 4096
BATCH = 2048
NSLOT = 8         # m-tile slots per core
NGRP = 4          # column groups
NT = 2            # n-tiles per batch half
NTW = 512         # n-tile width
KT = 32           # k tiles
NWC = 8           # W stream chunks (4 k-tiles each)
NXC = 4           # x chunks per n-tile (8 k-tiles each)

# k-major stream of (kt, slot) jobs; slot s is active for kt < 4(s+1)
STREAM = [(kt, s) for kt in range(KT) for s in range(NSLOT) if kt < 4 * (s + 1)]
T_IDX = {j: t for t, j in enumerate(STREAM)}
NSTREAM = len(STREAM)  # 144


def _kt_range(kt):
    """[t0, t1) tile-index range of stream tiles belonging to k-tile kt."""
    t0 = T_IDX[(kt, kt // 4)]
    return t0, t0 + NSLOT - kt // 4


def _wc_range(c):
    """[t0, t1) tile-index range of W chunk c (k-tiles 4c..4c+3)."""
    return _kt_range(4 * c)[0], _kt_range(4 * c + 3)[1]


def _build_program():
    import concourse.tile as tile
    from concourse import bacc, mybir

    F32 = mybir.dt.float32
    F32R = mybir.dt.float32r
    AF = mybir.ActivationFunctionType

    nc = bacc.Bacc("TRN2", target_bir_lowering=False, debug=False, num_devices=8)

    ws = nc.dram_tensor("ws", [128, NSTREAM * 128], F32R, kind="ExternalInput")
    # x, host-packed per-core in SBUF layout: [p, nt, xchunk, ktile-in-chunk, n]
    xt = nc.dram_tensor("xt", [128, NT, NXC, 8, NTW], F32R, kind="ExternalInput")
    xm = nc.dram_tensor("xm", [128, NT, NSLOT, NTW], F32R, kind="ExternalInput")
    dgt = nc.dram_tensor("dgt", [128, NSLOT, BLK], F32, kind="ExternalInput")
    dgr = nc.dram_tensor("dgr", [128, NSLOT, BLK], F32, kind="ExternalInput")
    b8 = nc.dram_tensor("b8", [128, NSLOT], F32, kind="ExternalInput")
    ls8 = nc.dram_tensor("ls8", [128, NSLOT], F32, kind="ExternalInput")
    on1 = nc.dram_tensor("on1", [128, 1], F32R, kind="ExternalInput")
    zz = nc.dram_tensor("zz", [128, NSLOT * 128], F32R, kind="ExternalInput")
    yt = nc.dram_tensor("yt", [1024, 1024], F32, kind="ExternalOutput")
    lj = nc.dram_tensor("lj", [128, NSLOT, BLK], F32, kind="ExternalOutput")
    srt = nc.dram_tensor("srt", [1024], F32)  # s roundtrip scratch

    with tile.TileContext(nc) as tc:
        with (
            tc.tile_pool(name="wp", bufs=1) as wp,
            tc.tile_pool(name="xp", bufs=5) as xp,
            tc.tile_pool(name="xmp", bufs=2) as xmp,
            tc.tile_pool(name="yp", bufs=4) as yp,
            tc.tile_pool(name="smallp", bufs=1) as smallp,
            tc.tile_pool(name="psy", bufs=6, space="PSUM") as pyp,
        ):
            # ---- small constants ----
            ones = smallp.tile([128, 1], F32R)
            nc.sync.dma_start(out=ones, in_=on1[:, :])
            dgt_sb = smallp.tile([128, NSLOT, BLK], F32)
            nc.sync.dma_start(out=dgt_sb, in_=dgt[:, :, :])
            dgr_sb = smallp.tile([128, NSLOT, BLK], F32)
            nc.sync.dma_start(out=dgr_sb, in_=dgr[:, :, :])
            b8_sb = smallp.tile([128, NSLOT], F32)
            nc.sync.dma_start(out=b8_sb, in_=b8[:, :])
            ls8_sb = smallp.tile([128, NSLOT], F32)
            nc.sync.dma_start(out=ls8_sb, in_=ls8[:, :])

            # ---- diag-block exp tiles (lhsT orientation) ----
            a_exp = smallp.tile([128, NSLOT, 128], F32R)
            nc.sync.dma_start(out=a_exp.rearrange("p s f -> p (s f)"), in_=zz[:, :])
            for s in range(NSLOT):
                for j in range(4):
                    nc.scalar.activation(
                        a_exp[32 * j : 32 * j + 32, s, 32 * j : 32 * j + 32],
                        dgt_sb[32 * j : 32 * j + 32, s, :],
                        AF.Exp,
                    )

            # x chunks for nt=0, requested up front so DMA overlaps W
            xc_sb = {}
            for c in range(NXC):
                xc = xp.tile([128, 8, NTW], F32R, tag="xc")
                nc.sync.dma_start(out=xc, in_=xt[:, 0, c, :, :])
                xc_sb[(0, c)] = xc

            w_chunk = []

            def w_tile(kt, s):
                t = T_IDX[(kt, s)]
                c = kt // 4
                t0, _ = _wc_range(c)
                return w_chunk[c][:, (t - t0) * 128 : (t - t0 + 1) * 128]

            def emit_matmuls(s, nt, out_sb, copy_raw):
                """Accumulate slot s, n-tile nt into PSUM, then copy to out_sb.

                copy_raw=True: plain PSUM->SBUF copy (g not available yet);
                the g*psum+bias affine is applied later in place.
                copy_raw=False: fused g*psum+bias copyback.
                """
                xms = xmp.tile([128, NTW], F32R, tag="xm", name=f"xm{nt}_{s}")
                nc.sync.dma_start(out=xms, in_=xm[:, nt, s, :])
                ps_y = pyp.tile([128, NTW], F32, tag="psy")
                for kt in range(4 * (s + 1)):
                    nc.tensor.matmul(
                        ps_y,
                        w_tile(kt, s),
                        xc_sb[(nt, kt // 8)][:, kt % 8, :],
                        start=(kt == 0),
                        stop=False,
                    )
                nc.tensor.matmul(ps_y, a_exp[:, s, :], xms, start=False, stop=True)
                if copy_raw:
                    nc.scalar.copy(out_sb, ps_y)
                else:
                    nc.scalar.activation(
                        out_sb, ps_y, AF.Identity,
                        bias=b8_sb[:, s : s + 1], scale=g_sb[:, s : s + 1],
                    )

            g_sb = smallp.tile([128, NSLOT], F32)
            ljb = smallp.tile([128, NSLOT], F32)
            # staging for nt=0 results (copied raw before g is known)
            stage = [
                smallp.tile([128, NTW], F32, tag=f"st{s}", name=f"stage{s}")
                for s in range(NSLOT)
            ]

            # ---- interleaved: W chunk DMA -> norm jobs -> main slot (nt=0) ----
            with (
                tc.tile_pool(name="psn", bufs=1, space="PSUM") as pnp,
                tc.tile_pool(name="sqp", bufs=2) as sqp,
            ):
                psn = pnp.tile([1, 1024], F32)
                for c in range(NWC):
                    t0c, t1c = _wc_range(c)
                    wc = wp.tile([128, (t1c - t0c) * 128], F32R, tag=f"wc{c}")
                    nc.sync.dma_start(out=wc, in_=ws[:, t0c * 128 : t1c * 128])
                    w_chunk.append(wc)
                    for kt in range(4 * c, 4 * c + 4):
                        t0, t1 = _kt_range(kt)
                        nact = t1 - t0
                        w_slice = wc[:, (t0 - t0c) * 128 : (t1 - t0c) * 128]
                        smin = c * 128
                        off = 0
                        while off < nact * 128:
                            n = min(512, nact * 128 - off)
                            sq = sqp.tile([128, 512], F32R, tag="sq")
                            nc.vector.tensor_mul(
                                sq[:, :n],
                                w_slice[:, off : off + n],
                                w_slice[:, off : off + n],
                            )
                            nc.tensor.matmul(
                                psn[0:1, smin + off : smin + off + n],
                                ones,
                                sq[:, :n],
                                start=(kt == 0),
                                stop=False,
                            )
                            off += n
                    # slot c (nt=0) needs only W chunks <= c and x chunks <= c//2
                    emit_matmuls(c, 0, stage[c], copy_raw=True)
                ae_flat = a_exp.rearrange("p s f -> p (s f)")
                for half in range(2):
                    a_sq = sqp.tile([128, 512], F32R, tag="sq", name=f"asq{half}")
                    nc.vector.tensor_mul(
                        a_sq, ae_flat[:, half * 512 : (half + 1) * 512],
                        ae_flat[:, half * 512 : (half + 1) * 512],
                    )
                    nc.tensor.matmul(
                        psn[0:1, half * 512 : (half + 1) * 512], ones, a_sq,
                        start=False, stop=(half == 1),
                    )
                s_row = smallp.tile([1, 1024], F32)
                nc.scalar.copy(s_row, psn[0:1, :])

            # s roundtrip through DRAM to get per-partition layout [128, 8]
            nc.sync.dma_start(out=srt[:].rearrange("(a n) -> a n", a=1), in_=s_row[0:1, :])
            s_col = smallp.tile([128, NSLOT], F32)
            nc.sync.dma_start(out=s_col, in_=srt[:].rearrange("(s p) -> p s", p=128))

            # g = exp(ls - 0.5 ln s);  ljb = ls - 0.5 ln s
            t_ln = smallp.tile([128, NSLOT], F32)
            nc.scalar.activation(t_ln, s_col, AF.Ln)
            for s in range(NSLOT):
                nc.scalar.activation(
                    ljb[:, s : s + 1], t_ln[:, s : s + 1], AF.Identity,
                    bias=ls8_sb[:, s : s + 1], scale=-0.5,
                )
            nc.scalar.activation(g_sb, ljb, AF.Exp)

            # log-jacobian: lj = W_diag + (ls - 0.5 ln s)
            lj_sb = smallp.tile([128, NSLOT, BLK], F32)
            for s in range(NSLOT):
                nc.scalar.activation(
                    lj_sb[:, s, :], dgr_sb[:, s, :], AF.Identity,
                    bias=ljb[:, s : s + 1], scale=1.0,
                )
            nc.sync.dma_start(out=lj[:, :, :], in_=lj_sb)

            # nt=0: apply y = g*acc + bias in place, then store
            for s in range(NSLOT):
                nc.scalar.activation(
                    stage[s], stage[s], AF.Identity,
                    bias=b8_sb[:, s : s + 1], scale=g_sb[:, s : s + 1],
                )
                nc.sync.dma_start(
                    out=yt[s * 128 : (s + 1) * 128, 0:NTW], in_=stage[s]
                )

            # ---- second batch half (nt=1) ----
            for c in range(NXC):
                xc = xp.tile([128, 8, NTW], F32R, tag="xc")
                nc.sync.dma_start(out=xc, in_=xt[:, 1, c, :, :])
                xc_sb[(1, c)] = xc
            for s in range(NSLOT):
                y_sb = yp.tile([128, NTW], F32, tag="y")
                emit_matmuls(s, 1, y_sb, copy_raw=False)
                nc.sync.dma_start(
                    out=yt[s * 128 : (s + 1) * 128, NTW : 2 * NTW], in_=y_sb
                )
    nc.finalize()
    return nc


def _pack_inputs(x, W, bias, W_log_scale):
    """Host-side data layout: slice/transpose/zero-select only (no math)."""
    f32 = np.float32
    WT = np.ascontiguousarray(W.T.astype(f32, copy=False))
    xT = np.ascontiguousarray(x.T.astype(f32, copy=False))
    bias = bias.astype(f32, copy=False)
    ls = W_log_scale.astype(f32, copy=False)

    grp = {}
    for g in range(NGRP):
        ws_g = np.zeros((128, NSTREAM, 128), f32)
        for t, (kt, s) in enumerate(STREAM):
            mg = 4 * s + g
            if kt < mg:
                ws_g[:, t, :] = WT[kt * 128 : (kt + 1) * 128, mg * 128 : (mg + 1) * 128]
            elif kt == mg:
                blk = WT[kt * 128 : (kt + 1) * 128, mg * 128 : (mg + 1) * 128].copy()
                for jp in range(4):  # keep only sub-blocks strictly above the diagonal
                    blk[32 * jp : 32 * jp + 32, : 32 * (jp + 1)] = 0.0
                ws_g[:, t, :] = blk
            # kt > mg: zero pad
        dgt = np.empty((128, NSLOT, BLK), f32)
        dgr = np.empty((128, NSLOT, BLK), f32)
        for s in range(NSLOT):
            mg = 4 * s + g
            blk = W[mg * 128 : (mg + 1) * 128, mg * 128 : (mg + 1) * 128]
            for j in range(4):
                sub = blk[32 * j : 32 * j + 32, 32 * j : 32 * j + 32]  # [a, b]
                dgt[32 * j : 32 * j + 32, s, :] = sub.T
                dgr[32 * j : 32 * j + 32, s, :] = sub
        b8 = np.stack(
            [bias[(4 * s + g) * 128 : (4 * s + g + 1) * 128] for s in range(NSLOT)], 1
        )
        ls8 = np.stack(
            [ls[(4 * s + g) * 128 : (4 * s + g + 1) * 128, 0] for s in range(NSLOT)], 1
        )
        grp[g] = dict(
            ws=np.ascontiguousarray(ws_g.reshape(128, NSTREAM * 128)),
            dgt=dgt, dgr=dgr, b8=np.ascontiguousarray(b8), ls8=np.ascontiguousarray(ls8),
        )

    # x in per-core SBUF layout: xt[p, nt, c, j, n] = xT[(8c+j)*128+p, h*1024+nt*512+n]
    xt_h = {}
    xm_h = {}
    for h in range(2):
        xs = xT[:, h * 1024 : (h + 1) * 1024]               # [4096, 1024]
        v = xs.reshape(KT, 128, NT, NTW)                     # [kt, p, nt, n]
        v = v.transpose(1, 2, 0, 3)                          # [p, nt, kt, n]
        xt_h[h] = np.ascontiguousarray(v.reshape(128, NT, NXC, 8, NTW))
    in_maps = []
    for c in range(8):
        g, h = c // 2, c % 2
        xm_c = np.empty((128, NT, NSLOT, NTW), f32)
        for s in range(NSLOT):
            mg = 4 * s + g
            for nt in range(NT):
                xm_c[:, nt, s, :] = xT[
                    mg * 128 : (mg + 1) * 128,
                    h * 1024 + nt * NTW : h * 1024 + (nt + 1) * NTW,
                ]
        in_maps.append(
            {
                **grp[g],
                "xt": xt_h[h],
                "xm": xm_c,
                "on1": np.ones((128, 1), f32),
                "zz": np.zeros((128, NSLOT * 128), f32),
            }
        )
    return in_maps


_PROGRAM = None


def kernel(x, W, bias, W_log_scale, b_diag_mask=None, b_tril_mask=None, **_ignored):
    """Full inputs in, full outputs out. Returns (y, log_jac) like the reference.

    The diag/tril masks are the fixed block-kronecker patterns of the module
    (block-diagonal + strict block-lower-triangular); the kernel exploits that
    structure directly, so the mask tensors themselves are not transferred.
    """
    from concourse.bass_utils import run_bass_kernel_spmd

    global _PROGRAM
    if _PROGRAM is None:
        _PROGRAM = _build_program()
    nc = _PROGRAM

    in_maps = _pack_inputs(
        np.asarray(x), np.asarray(W), np.asarray(bias), np.asarray(W_log_scale)
    )
    res = run_bass_kernel_spmd(nc, in_maps, list(range(8)))

    y = np.empty((BATCH, DIM), np.float32)
    lj_full = np.empty((NB, BLK, BLK), np.float32)
    for c, r in enumerate(res.results):
        g, h = c // 2, c % 2
        ytc = r["yt"]
        for s in range(NSLOT):
            mg = 4 * s + g
            y[h * 1024 : (h + 1) * 1024, mg * 128 : (mg + 1) * 128] = (
                ytc[s * 128 : (s + 1) * 128, :].T
            )
        if h == 0:
            ljc = r["lj"]  # [128, 8, 32]
            for s in range(NSLOT):
                mg = 4 * s + g
                for j in range(4):
                    lj_full[4 * mg + j] = ljc[32 * j : 32 * j + 32, s, :]
    return (y, lj_full)
